# revision 20
# baseline (speedup 1.0000x reference)
"""AdaptiveBlockSelfAttention Trainium2 kernel (8-core SPMD, no collectives).

Problem: x[2,256,192,192]; 1x1-conv QKV projections; block-local attention
within 16x16 spatial blocks (8 heads, d=32); output projection + residual;
LayerNorm over channels.

Sharding: the 24 (batch, block-row) slabs are data-parallel -> 3 slabs/core
on 8 cores. Weights replicated. Everything else is core-local.

This core is PE-duty-cycle limited (HAM power throttle caps the tensor
engine near K=4/8 on 8-core SPMD), so the design minimizes PE busy-cycles
and keeps the in-order PE queue free of head-of-line stalls:
  - Q^T,K^T channel-major [128, 2, 256] per proj: head h = 4*ot + a sits
    at partitions [32a, 32a+32) of half ot -- a direct PSUM copy, and
    exactly the base partitions needed for 4-way row-tiled score matmuls.
  - Scores S^T (K=32) run 4 heads concurrently via tile_position=(32a, 0).
  - A^T = exp(S^T*scale) on ACT (the only ACT work besides qkt copies).
  - AV (M=32, no ones column) runs 4 heads concurrently via
    tile_position=(0, 32a); outputs for heads 4ot..4ot+3 land as the exact
    channel-major block [128ot:128ot+128] x [256 px].
  - Softmax denominators via separate col-tiled matmuls with lhsT =
    ones[128, 32]: each head's row-sum lands broadcast across its 32
    channel partitions -> one full-width reciprocal + one multiply per
    half, no per-head scalar chain, no gpsimd broadcasts.
  - Wo projection pixel-major (oc-stationary) + residual via PE
    transpose-accumulate of X into the same PSUM group (transposes first
    so the PE isn't waiting on the softmax-normalize DVE chain).
  - LayerNorm with bn_stats/bn_aggr; rstd via magic-constant Newton (DVE).
  - PE transpose back to channel-major, copy into the output slab.
  - All PSUM tiles come from one rotating 8-bank pool (tile = 1 bank).
  - Unit emission is software-pipelined: front(j+1) [QKV/scores/exp] is
    emitted before back(j) [AV/Wo/LN] so the PE queue always has
    independent work behind a stalled instruction.
"""

import numpy as np

import concourse.bacc as bacc
import concourse.tile as tile
import concourse.mybir as mybir
from concourse.bass_utils import run_bass_kernel_spmd
from concourse.masks import make_identity

F32 = mybir.dt.float32
F32R = mybir.dt.float32r
BF16 = mybir.dt.bfloat16
FP8 = mybir.dt.float8e4
I32 = mybir.dt.int32
PM = mybir.MatmulPerfMode
AF = mybir.ActivationFunctionType
ALU = mybir.AluOpType

N_CORES = 8
C = 256
HW_ = 192
BS = 16
NH = 8
D = 32
EPS = 1e-5
SCALE = float(1.0 / np.sqrt(D))
G_X = 16.0            # host scale on fp8 x
S_W = 16.0            # host scale on fp8 Wq/Wk/Wv
S_WO = 16.0           # host scale on fp8 Wo (residual x_res carries the
                      # matching 16x; LayerNorm is affine-invariant per pixel)
EXP_SCALE = float(SCALE / (G_X * S_W) ** 2)
OC_SCALE = float(1.0 / (G_X * S_W))

N_SLABS = 3          # block-rows per core
N_UNITS = 12         # 16x16 blocks per block-row


def _build(apply_gb: bool, n_slabs: int = N_SLABS, n_units: int = N_UNITS):
    nc = bacc.Bacc("TRN2", target_bir_lowering=False, debug=False)

    x_ext = nc.declare_dram_parameter("x", [n_slabs, C, N_UNITS, 256], F32R, isOutput=False)
    x8_ext = nc.declare_dram_parameter("x8", [n_slabs, C, N_UNITS, 256], FP8, isOutput=False)
    out_ext = nc.declare_dram_parameter("out", [n_slabs, N_UNITS, 256, C], F32, isOutput=True)
    wq_ext = nc.declare_dram_parameter("wqt", [C, C], FP8, isOutput=False)
    wk_ext = nc.declare_dram_parameter("wkt", [C, C], FP8, isOutput=False)
    wv_ext = nc.declare_dram_parameter("wvt", [C, C], FP8, isOutput=False)
    wo_ext = nc.declare_dram_parameter("wot", [C, C], FP8, isOutput=False)
    gamma_ext = nc.declare_dram_parameter("gamma", [1, C], F32, isOutput=False)
    beta_ext = nc.declare_dram_parameter("beta", [1, C], F32, isOutput=False)

    with tile.TileContext(nc) as tc:
        with (
            tc.tile_pool(name="consts", bufs=1) as consts,
            tc.tile_pool(name="xin", bufs=2) as p_xin,
            tc.tile_pool(name="x8in", bufs=2) as p_x8in,
            tc.tile_pool(name="xu", bufs=3) as p_xu,
            tc.tile_pool(name="xout", bufs=2) as p_xout,
            tc.tile_pool(name="qksb", bufs=3) as p_qk,
            tc.tile_pool(name="vsb", bufs=4) as p_v,
            tc.tile_pool(name="atsb", bufs=26) as p_at,
            tc.tile_pool(name="ocsb", bufs=2) as p_oc,
            tc.tile_pool(name="recsb", bufs=2) as p_rec,
            tc.tile_pool(name="usb", bufs=2) as p_u,
            tc.tile_pool(name="small", bufs=8) as p_small,
            tc.tile_pool(name="psf", bufs=5, space="PSUM") as psf,
            tc.tile_pool(name="psav", bufs=2, space="PSUM") as psav,
            tc.tile_pool(name="pspt", bufs=1, space="PSUM") as pspt,
        ):
            # ---- constants ----
            ident_f32 = consts.tile([128, 128], F32, tag="ident_f32")
            make_identity(nc, ident_f32[:])
            ident = consts.tile([128, 128], F32R)
            nc.vector.tensor_copy(out=ident[:], in_=ident_f32[:])
            ones32 = consts.tile([128, D], BF16, tag="ones32")
            nc.vector.memset(ones32[:], 1.0)
            magic_sb = consts.tile([128, 2], I32, tag="magic")
            nc.vector.memset(magic_sb[:], 0x5F3759DF)

            w_sbs = {}
            for nm, ext in (("wq", wq_ext), ("wk", wk_ext), ("wv", wv_ext), ("wo", wo_ext)):
                w_sb = consts.tile([128, 2, C], FP8, tag=nm)
                nc.sync.dma_start(out=w_sb[:], in_=ext[:].rearrange("(t p) o -> p t o", p=128))
                w_sbs[nm] = w_sb
            wq_sb, wk_sb, wv_sb, wo_sb = (w_sbs[n] for n in ("wq", "wk", "wv", "wo"))

            if apply_gb:
                g_row = consts.tile([1, C], F32, tag="g_row")
                b_row = consts.tile([1, C], F32, tag="b_row")
                nc.sync.dma_start(out=g_row[:], in_=gamma_ext[:])
                nc.sync.dma_start(out=b_row[:], in_=beta_ext[:])
                G128 = consts.tile([128, C], F32, tag="G128")
                B128 = consts.tile([128, C], F32, tag="B128")
                nc.gpsimd.partition_broadcast(out_ap=G128[:], in_ap=g_row[:])
                nc.gpsimd.partition_broadcast(out_ap=B128[:], in_ap=b_row[:])

            slab_sbs = {}

            def emit_front(s, j):
                # unit view of fp8 X: [c, kt, 256 px] contiguous
                x8v = slab_sbs[s]["x8"][:, :, j, :]

                # ---- Q^T, K^T channel-major (fp8 DoubleRow, K=256/pass);
                # head h = 4*ot + a at partitions [32a, 32a+32) of half ot ----
                qk_q = psf.tile([128, 2, 256], F32, tag="psf")
                qk_k = psf.tile([128, 2, 256], F32, tag="psf")
                for qk_ps, w_sb in ((qk_q, wq_sb), (qk_k, wk_sb)):
                    for ot in range(2):
                        nc.tensor.matmul(
                            out=qk_ps[:, ot, :],
                            lhsT=w_sb[:, :, 128 * ot:128 * ot + 128],
                            rhs=x8v[:, :, :],
                            start=True, stop=True,
                            perf_mode=PM.DoubleRow,
                        )
                qkt_sb = p_qk.tile([128, 4, 256], BF16, tag="qkt")
                nc.scalar.activation(out=qkt_sb[:, 0:2, :], in_=qk_q[:], func=AF.Copy)
                nc.scalar.activation(out=qkt_sb[:, 2:4, :], in_=qk_k[:], func=AF.Copy)

                # ---- V pixel-major [j, 2(jt), 8 heads, 32] (fp8 DoubleRow) ----
                v_ps = psf.tile([128, 2, 256], F32, tag="psf")
                for pt in range(2):
                    nc.tensor.matmul(
                        out=v_ps[:, pt, :],
                        lhsT=x8v[:, :, 128 * pt:128 * pt + 128],
                        rhs=wv_sb[:, :, :],
                        start=True, stop=True,
                        perf_mode=PM.DoubleRow,
                    )
                v_sb = p_v.tile([128, 2, NH, D], BF16, tag="v_sb")
                nc.vector.tensor_copy(
                    out=v_sb[:].rearrange("p t h d -> p t (h d)"), in_=v_ps[:]
                )

                # ---- scores: 4-way row-tiled S^T per half ot; exp on ACT ----
                at_sbs = [None] * NH
                for ot in range(2):
                    st_a = []
                    for _a in range(4):
                        st_t = psf.tile([128, 2, 256], F32, tag="psf")
                        st_a.append(st_t)
                    for jt in range(2):
                        for a in range(4):
                            nc.tensor.matmul(
                                out=st_a[a][:, jt, :],
                                lhsT=qkt_sb[32 * a:32 * a + D, 2 + ot, 128 * jt:128 * jt + 128],
                                rhs=qkt_sb[32 * a:32 * a + D, ot, :],
                                start=True, stop=True,
                                tile_position=(32 * a, 0),
                                skip_group_check=True,
                            )
                    for a in range(4):
                        h = 4 * ot + a
                        at_sb = p_at.tile([128, 2, 256], BF16, tag="at")
                        nc.scalar.activation(
                            out=at_sb[:], in_=st_a[a][:], func=AF.Exp, scale=EXP_SCALE
                        )
                        at_sbs[h] = at_sb

                return {"s": s, "j": j, "v_sb": v_sb, "at": at_sbs}

            def emit_mid(st):
                s, j = st["s"], st["j"]
                v_sb, at_sbs = st["v_sb"], st["at"]
                xv = slab_sbs[s]["x"][:, :, j, :]

                # ---- AV (col-tiled 4-way) + denominators ----
                otu = psav.tile([128, 2, 256], F32, tag="psav")
                lden = psav.tile([128, 2, 256], F32, tag="psav")
                rec_sb = p_rec.tile([128, 2, 256], F32, tag="rec")
                oc_sb = p_oc.tile([128, 2, 256], FP8, tag="oc")
                for ot in range(2):
                    for jt in range(2):
                        for a in range(4):
                            h = 4 * ot + a
                            nc.tensor.matmul(
                                out=otu[32 * a:32 * a + D, ot, :],
                                lhsT=v_sb[:, jt, h, :],
                                rhs=at_sbs[h][:, jt, :],
                                start=(jt == 0), stop=(jt == 1),
                                tile_position=(0, 32 * a),
                                skip_group_check=True,
                            )
                    for jt in range(2):
                        for a in range(4):
                            h = 4 * ot + a
                            nc.tensor.matmul(
                                out=lden[32 * a:32 * a + D, ot, :],
                                lhsT=ones32[:],
                                rhs=at_sbs[h][:, jt, :],
                                start=(jt == 0), stop=(jt == 1),
                                tile_position=(0, 32 * a),
                                skip_group_check=True,
                            )
                    # normalize: oc = otu * (1/l), channel-major bf16
                    nc.vector.reciprocal_approx_fast(
                        out=rec_sb[:, ot, :], in_=lden[:, ot, :]
                    )
                    nc.vector.scalar_tensor_tensor(
                        out=oc_sb[:, ot, :], in0=otu[:, ot, :], scalar=OC_SCALE,
                        in1=rec_sb[:, ot, :], op0=ALU.mult, op1=ALU.mult,
                    )

                # ---- residual transpose-accumulate + Wo projection ----
                # (transposes first: they only need xu, so the PE isn't
                # stalled on the recip/mult chain producing oc)
                pt_ps = pspt.tile([128, 2, 256], F32, tag="pspt")
                for pt in range(2):
                    for ct in range(2):
                        nc.tensor.matmul(
                            out=pt_ps[:, pt, 128 * ct:128 * ct + 128].bitcast(F32R),
                            lhsT=xv[:, ct, 128 * pt:128 * pt + 128],
                            rhs=ident[:],
                            is_transpose=True, start=(ct == 0), stop=False,
                            skip_group_check=True,
                        )
                    nc.tensor.matmul(
                        out=pt_ps[:, pt, :],
                        lhsT=oc_sb[:, :, 128 * pt:128 * pt + 128],
                        rhs=wo_sb[:, :, :],
                        start=False, stop=True,
                        perf_mode=PM.DoubleRow,
                        skip_group_check=True,
                    )

                # ---- LayerNorm (free axis = channels), written
                # directly into the pixel-major out slab ----
                mv2 = p_small.tile([128, 2, 2], F32, tag="mv2")
                for pt in range(2):
                    stats = p_small.tile([128, 6], F32, tag="stats")
                    nc.vector.bn_stats(out=stats[:], in_=pt_ps[:, pt, :])
                    nc.vector.bn_aggr(out=mv2[:, pt, :], in_=stats[:])
                # rstd = 1/sqrt(var+eps) via magic-constant + two Newton steps (DVE)
                ve = p_small.tile([128, 2], F32, tag="ve")
                nc.vector.tensor_scalar(out=ve[:], in0=mv2[:, :, 1], scalar1=EPS,
                                        scalar2=None, op0=ALU.add)
                hbits = p_small.tile([128, 2], I32, tag="hbits")
                nc.vector.tensor_scalar(out=hbits[:], in0=ve[:].bitcast(I32),
                                        scalar1=1, scalar2=None, op0=ALU.arith_shift_right)
                y0 = p_small.tile([128, 2], F32, tag="y0")
                nc.vector.tensor_tensor(out=y0[:].bitcast(I32), in0=magic_sb[:],
                                        in1=hbits[:], op=ALU.subtract)
                a_t = p_small.tile([128, 2], F32, tag="a_t")
                nc.vector.tensor_tensor(out=a_t[:], in0=ve[:], in1=y0[:], op=ALU.mult)
                nc.vector.tensor_tensor(out=a_t[:], in0=a_t[:], in1=y0[:], op=ALU.mult)
                nc.vector.tensor_scalar(out=a_t[:], in0=a_t[:], scalar1=-0.5, scalar2=1.5,
                                        op0=ALU.mult, op1=ALU.add)
                rstd2 = p_small.tile([128, 2], F32, tag="rstd2")
                nc.vector.tensor_tensor(out=rstd2[:], in0=y0[:], in1=a_t[:], op=ALU.mult)
                b_t = p_small.tile([128, 2], F32, tag="b_t")
                nc.vector.tensor_tensor(out=b_t[:], in0=ve[:], in1=rstd2[:], op=ALU.mult)
                nc.vector.tensor_tensor(out=b_t[:], in0=b_t[:], in1=rstd2[:], op=ALU.mult)
                nc.vector.tensor_scalar(out=b_t[:], in0=b_t[:], scalar1=-0.5, scalar2=1.5,
                                        op0=ALU.mult, op1=ALU.add)
                nc.vector.tensor_tensor(out=rstd2[:], in0=rstd2[:], in1=b_t[:], op=ALU.mult)
                nmr2 = p_small.tile([128, 2], F32, tag="nmr2")
                nc.vector.scalar_tensor_tensor(
                    out=nmr2[:], in0=mv2[:, :, 0], scalar=-1.0, in1=rstd2[:],
                    op0=ALU.mult, op1=ALU.mult,
                )
                out_sb = slab_sbs[s]["out"]
                for pt in range(2):
                    nc.vector.tensor_scalar(
                        out=out_sb[:, j, pt, :], in0=pt_ps[:, pt, :],
                        scalar1=rstd2[:, pt:pt + 1], scalar2=nmr2[:, pt:pt + 1],
                        op0=ALU.mult, op1=ALU.add,
                    )
                    if apply_gb:
                        nc.vector.tensor_tensor(
                            out=out_sb[:, j, pt, :], in0=out_sb[:, j, pt, :],
                            in1=G128[:], op=ALU.mult
                        )
                        nc.vector.tensor_tensor(
                            out=out_sb[:, j, pt, :], in0=out_sb[:, j, pt, :],
                            in1=B128[:], op=ALU.add
                        )
                return st

            def emit_tail(st):
                pass

            total = n_slabs * n_units
            f_states = [None, None]
            for idx in range(total + 2):
                if idx < total:
                    s, j = divmod(idx, n_units)
                    if j == 0 and s == 0:
                        x8_sb = p_x8in.tile([128, 2, N_UNITS, 256], FP8, tag="x8_sb")
                        nc.sync.dma_start(
                            out=x8_sb[:],
                            in_=x8_ext[s].rearrange("(t p) u i -> p t u i", p=128),
                        )
                        x_sb = p_xin.tile([128, 2, N_UNITS, 256], F32R, tag="x_sb")
                        nc.sync.dma_start(
                            out=x_sb[:],
                            in_=x_ext[s].rearrange("(t p) u i -> p t u i", p=128),
                        )
                        slab_sbs[s] = {"x": x_sb, "x8": x8_sb}
                    if j == 0:
                        out_sb = p_xout.tile([128, N_UNITS, 2, C], F32, tag="out_sb")
                        if n_units < N_UNITS:
                            nc.vector.memset(out_sb[:], 0.0)
                        slab_sbs[s]["out"] = out_sb
                    if j == n_units // 2 and s + 1 < n_slabs:
                        x8_sb = p_x8in.tile([128, 2, N_UNITS, 256], FP8, tag="x8_sb")
                        nc.sync.dma_start(
                            out=x8_sb[:],
                            in_=x8_ext[s + 1].rearrange("(t p) u i -> p t u i", p=128),
                        )
                        x_sb = p_xin.tile([128, 2, N_UNITS, 256], F32R, tag="x_sb")
                        nc.sync.dma_start(
                            out=x_sb[:],
                            in_=x_ext[s + 1].rearrange("(t p) u i -> p t u i", p=128),
                        )
                        slab_sbs[s + 1] = {"x": x_sb, "x8": x8_sb}
                    fs = emit_front(s, j)
                else:
                    fs = None
                old = f_states.pop(0)
                f_states.append(fs)
                if old is not None:
                    emit_mid(old)
                    if old["j"] == n_units - 1:
                        ps_ = old["s"]
                        nc.sync.dma_start(
                            out=out_ext[ps_].rearrange("u (t p) c -> p u t c", p=128),
                            in_=slab_sbs[ps_]["out"][:],
                        )

    nc.compile()
    return nc


_CACHE = {}


def _get(apply_gb: bool):
    if apply_gb not in _CACHE:
        _CACHE[apply_gb] = _build(apply_gb)
    return _CACHE[apply_gb]


def _in_maps(x, Wq, Wk, Wv, Wo, gamma, beta):
    import ml_dtypes
    E4M3 = ml_dtypes.float8_e4m3fn
    x = np.ascontiguousarray(x, dtype=np.float32)
    B = x.shape[0]
    xr = x.reshape(B, C, 12, BS, 12, BS).transpose(0, 2, 1, 4, 3, 5).reshape(B * 12, C, 12, BS * BS)
    xres = np.ascontiguousarray(xr * np.float32(S_WO))
    x8 = np.ascontiguousarray((xr * np.float32(G_X)).astype(E4M3))
    wqt = np.ascontiguousarray((np.asarray(Wq, dtype=np.float32).T * np.float32(S_W)).astype(E4M3))
    wkt = np.ascontiguousarray((np.asarray(Wk, dtype=np.float32).T * np.float32(S_W)).astype(E4M3))
    wvt = np.ascontiguousarray((np.asarray(Wv, dtype=np.float32).T * np.float32(S_W)).astype(E4M3))
    wot = np.ascontiguousarray((np.asarray(Wo, dtype=np.float32).T * np.float32(S_WO)).astype(E4M3))
    g = np.ascontiguousarray(np.asarray(gamma, dtype=np.float32).reshape(1, C))
    b = np.ascontiguousarray(np.asarray(beta, dtype=np.float32).reshape(1, C))
    maps = []
    for core in range(N_CORES):
        sl = slice(core * N_SLABS, (core + 1) * N_SLABS)
        maps.append({
            "x": np.ascontiguousarray(xres[sl]),
            "x8": np.ascontiguousarray(x8[sl]),
            "wqt": wqt, "wkt": wkt, "wvt": wvt, "wot": wot,
            "gamma": g, "beta": b,
        })
    return maps


def _assemble(results, B=2):
    outs = np.stack([results[i]["out"] for i in range(N_CORES)])
    # outs: [cores, ns, 12u, 256px, C] pixel-major
    o = outs.reshape(B, 12, 12, BS, BS, C).transpose(0, 5, 1, 3, 2, 4).reshape(B, C, 12 * BS, HW_)
    return np.ascontiguousarray(o)


def run(x, Wq, Wk, Wv, Wo, gamma, beta, **spmd_kwargs):
    gamma = np.asarray(gamma, dtype=np.float32)
    beta = np.asarray(beta, dtype=np.float32)
    apply_gb = not (np.allclose(gamma, 1.0) and np.all(beta == 0.0))
    nc = _get(apply_gb)
    maps = _in_maps(x, Wq, Wk, Wv, Wo, gamma, beta)
    res = run_bass_kernel_spmd(nc, maps, core_ids=list(range(N_CORES)), **spmd_kwargs)
    return _assemble(res.results, B=np.asarray(x).shape[0]), res


def kernel(x, Wq, Wk, Wv, Wo, gamma, beta):
    out, _ = run(x, Wq, Wk, Wv, Wo, gamma, beta)
    return out


# revision 21
# speedup vs baseline: 1.2248x; 1.2248x over previous
"""AdaptiveBlockSelfAttention Trainium2 kernel (8-core SPMD, no collectives).

Problem: x[2,256,192,192]; 1x1-conv QKV projections; block-local attention
within 16x16 spatial blocks (8 heads, d=32); output projection + residual;
LayerNorm over channels.

Sharding: the 24 (batch, block-row) slabs are data-parallel -> 3 slabs/core
on 8 cores. Weights replicated. Everything else is core-local.

This core is PE-duty-cycle limited (HAM power throttle caps the tensor
engine near K=4/8 on 8-core SPMD), so the design minimizes PE busy-cycles
and keeps the in-order PE queue free of head-of-line stalls:
  - Q^T,K^T channel-major [128, 2, 256] per proj: head h = 4*ot + a sits
    at partitions [32a, 32a+32) of half ot -- a direct PSUM copy, and
    exactly the base partitions needed for 4-way row-tiled score matmuls.
  - Scores S^T (K=32) run 4 heads concurrently via tile_position=(32a, 0).
  - A^T = exp(S^T*scale) on ACT (the only ACT work besides qkt copies).
  - AV (M=32, no ones column) runs 4 heads concurrently via
    tile_position=(0, 32a); outputs for heads 4ot..4ot+3 land as the exact
    channel-major block [128ot:128ot+128] x [256 px].
  - Softmax denominators via separate col-tiled matmuls with lhsT =
    ones[128, 32]: each head's row-sum lands broadcast across its 32
    channel partitions -> one full-width reciprocal + one multiply per
    half, no per-head scalar chain, no gpsimd broadcasts.
  - Wo projection pixel-major (oc-stationary) + residual via PE
    transpose-accumulate of X into the same PSUM group (transposes first
    so the PE isn't waiting on the softmax-normalize DVE chain).
  - LayerNorm with bn_stats/bn_aggr; rstd via magic-constant Newton (DVE).
  - PE transpose back to channel-major, copy into the output slab.
  - All PSUM tiles come from one rotating 8-bank pool (tile = 1 bank).
  - Unit emission is software-pipelined: front(j+1) [QKV/scores/exp] is
    emitted before back(j) [AV/Wo/LN] so the PE queue always has
    independent work behind a stalled instruction.
"""

import numpy as np

import concourse.bacc as bacc
import concourse.tile as tile
import concourse.mybir as mybir
from concourse.bass_utils import run_bass_kernel_spmd
from concourse.masks import make_identity

F32 = mybir.dt.float32
F32R = mybir.dt.float32r
BF16 = mybir.dt.bfloat16
FP8 = mybir.dt.float8e4
I32 = mybir.dt.int32
PM = mybir.MatmulPerfMode
AF = mybir.ActivationFunctionType
ALU = mybir.AluOpType

N_CORES = 8
C = 256
HW_ = 192
BS = 16
NH = 8
D = 32
EPS = 1e-5
SCALE = float(1.0 / np.sqrt(D))
G_X = 16.0            # host scale on fp8 x
S_W = 16.0            # host scale on fp8 Wq/Wk/Wv
S_WO = 16.0           # host scale on fp8 Wo (residual x_res carries the
                      # matching 16x; LayerNorm is affine-invariant per pixel)
EXP_SCALE = float(SCALE / (G_X * S_W) ** 2)
OC_SCALE = float(1.0 / (G_X * S_W))

N_SLABS = 3          # block-rows per core
N_UNITS = 12         # 16x16 blocks per block-row


def _build(apply_gb: bool, n_slabs: int = N_SLABS, n_units: int = N_UNITS):
    nc = bacc.Bacc("TRN2", target_bir_lowering=False, debug=False)

    x_ext = nc.declare_dram_parameter("x", [n_slabs, C, N_UNITS, 256], F32R, isOutput=False)
    x8_ext = nc.declare_dram_parameter("x8", [n_slabs, C, N_UNITS, 256], FP8, isOutput=False)
    out_ext = nc.declare_dram_parameter("out", [n_slabs, N_UNITS, 256, C], F32, isOutput=True)
    wq_ext = nc.declare_dram_parameter("wqt", [C, C], FP8, isOutput=False)
    wk_ext = nc.declare_dram_parameter("wkt", [C, C], FP8, isOutput=False)
    wv_ext = nc.declare_dram_parameter("wvt", [C, C], FP8, isOutput=False)
    wo_ext = nc.declare_dram_parameter("wot", [C, C], FP8, isOutput=False)
    gamma_ext = nc.declare_dram_parameter("gamma", [1, C], F32, isOutput=False)
    beta_ext = nc.declare_dram_parameter("beta", [1, C], F32, isOutput=False)

    with tile.TileContext(nc) as tc:
        with (
            tc.tile_pool(name="consts", bufs=1) as consts,
            tc.tile_pool(name="xin", bufs=2) as p_xin,
            tc.tile_pool(name="x8in", bufs=2) as p_x8in,
            tc.tile_pool(name="xu", bufs=3) as p_xu,
            tc.tile_pool(name="xout", bufs=2) as p_xout,
            tc.tile_pool(name="qksb", bufs=3) as p_qk,
            tc.tile_pool(name="vsb", bufs=4) as p_v,
            tc.tile_pool(name="atsb", bufs=26) as p_at,
            tc.tile_pool(name="ocsb", bufs=2) as p_oc,
            tc.tile_pool(name="recsb", bufs=2) as p_rec,
            tc.tile_pool(name="usb", bufs=2) as p_u,
            tc.tile_pool(name="small", bufs=8) as p_small,
            tc.tile_pool(name="psf", bufs=5, space="PSUM") as psf,
            tc.tile_pool(name="psav", bufs=2, space="PSUM") as psav,
            tc.tile_pool(name="pspt", bufs=1, space="PSUM") as pspt,
        ):
            # ---- constants ----
            ident_f32 = consts.tile([128, 128], F32, tag="ident_f32")
            make_identity(nc, ident_f32[:])
            ident = consts.tile([128, 128], F32R)
            nc.vector.tensor_copy(out=ident[:], in_=ident_f32[:])
            ones32 = consts.tile([128, D], BF16, tag="ones32")
            nc.vector.memset(ones32[:], 1.0)
            magic_sb = consts.tile([128, 2], I32, tag="magic")
            nc.vector.memset(magic_sb[:], 0x5F3759DF)

            w_sbs = {}
            for nm, ext in (("wq", wq_ext), ("wk", wk_ext), ("wv", wv_ext), ("wo", wo_ext)):
                w_sb = consts.tile([128, 2, C], FP8, tag=nm)
                nc.sync.dma_start(out=w_sb[:], in_=ext[:].rearrange("(t p) o -> p t o", p=128))
                w_sbs[nm] = w_sb
            wq_sb, wk_sb, wv_sb, wo_sb = (w_sbs[n] for n in ("wq", "wk", "wv", "wo"))

            if apply_gb:
                g_row = consts.tile([1, C], F32, tag="g_row")
                b_row = consts.tile([1, C], F32, tag="b_row")
                nc.sync.dma_start(out=g_row[:], in_=gamma_ext[:])
                nc.sync.dma_start(out=b_row[:], in_=beta_ext[:])
                G128 = consts.tile([128, C], F32, tag="G128")
                B128 = consts.tile([128, C], F32, tag="B128")
                nc.gpsimd.partition_broadcast(out_ap=G128[:], in_ap=g_row[:])
                nc.gpsimd.partition_broadcast(out_ap=B128[:], in_ap=b_row[:])

            slab_sbs = {}

            def emit_st_round(qkt_sb, at_sbs, ot):
                st_a = []
                for _a in range(4):
                    st_t = psf.tile([128, 2, 256], F32, tag="psf")
                    st_a.append(st_t)
                for jt in range(2):
                    for a in range(4):
                        nc.tensor.matmul(
                            out=st_a[a][:, jt, :],
                            lhsT=qkt_sb[32 * a:32 * a + D, 2 + ot, 128 * jt:128 * jt + 128],
                            rhs=qkt_sb[32 * a:32 * a + D, ot, :],
                            start=True, stop=True,
                            tile_position=(32 * a, 0),
                            skip_group_check=True,
                        )
                for a in range(4):
                    h = 4 * ot + a
                    at_sb = p_at.tile([128, 2, 256], BF16, tag="at")
                    nc.scalar.activation(
                        out=at_sb[:], in_=st_a[a][:], func=AF.Exp, scale=EXP_SCALE
                    )
                    at_sbs[h] = at_sb

            def emit_frontA(s, j):
                # unit view of fp8 X: [c, kt, 256 px] contiguous
                x8v = slab_sbs[s]["x8"][:, :, j, :]

                # ---- Q^T, K^T channel-major (fp8 DoubleRow, K=256/pass);
                # head h = 4*ot + a at partitions [32a, 32a+32) of half ot ----
                qk_q = psf.tile([128, 2, 256], F32, tag="psf")
                qk_k = psf.tile([128, 2, 256], F32, tag="psf")
                for qk_ps, w_sb in ((qk_q, wq_sb), (qk_k, wk_sb)):
                    for ot in range(2):
                        nc.tensor.matmul(
                            out=qk_ps[:, ot, :],
                            lhsT=w_sb[:, :, 128 * ot:128 * ot + 128],
                            rhs=x8v[:, :, :],
                            start=True, stop=True,
                            perf_mode=PM.DoubleRow,
                        )
                qkt_sb = p_qk.tile([128, 4, 256], BF16, tag="qkt")
                nc.scalar.activation(out=qkt_sb[:, 0:2, :], in_=qk_q[:], func=AF.Copy)
                nc.scalar.activation(out=qkt_sb[:, 2:4, :], in_=qk_k[:], func=AF.Copy)

                at_sbs = [None] * NH
                emit_st_round(qkt_sb, at_sbs, 0)
                return {"s": s, "j": j, "qkt": qkt_sb, "at": at_sbs}

            def emit_frontB(st):
                s, j = st["s"], st["j"]
                emit_st_round(st["qkt"], st["at"], 1)
                # ---- V pixel-major [j, 2(jt), 8 heads, 32] (fp8 DoubleRow) ----
                x8v = slab_sbs[s]["x8"][:, :, j, :]
                v_ps = psf.tile([128, 2, 256], F32, tag="psf")
                for pt in range(2):
                    nc.tensor.matmul(
                        out=v_ps[:, pt, :],
                        lhsT=x8v[:, :, 128 * pt:128 * pt + 128],
                        rhs=wv_sb[:, :, :],
                        start=True, stop=True,
                        perf_mode=PM.DoubleRow,
                    )
                v_sb = p_v.tile([128, 2, NH, D], BF16, tag="v_sb")
                nc.vector.tensor_copy(
                    out=v_sb[:].rearrange("p t h d -> p t (h d)"), in_=v_ps[:]
                )
                st["v_sb"] = v_sb

            def emit_mid(st):
                s, j = st["s"], st["j"]
                v_sb, at_sbs = st["v_sb"], st["at"]
                xv = slab_sbs[s]["x"][:, :, j, :]

                # ---- AV (col-tiled 4-way) + denominators ----
                otu = psav.tile([128, 2, 256], F32, tag="psav")
                lden = psav.tile([128, 2, 256], F32, tag="psav")
                rec_sb = p_rec.tile([128, 2, 256], F32, tag="rec")
                oc_sb = p_oc.tile([128, 2, 256], FP8, tag="oc")
                for ot in range(2):
                    for jt in range(2):
                        for a in range(4):
                            h = 4 * ot + a
                            nc.tensor.matmul(
                                out=otu[32 * a:32 * a + D, ot, :],
                                lhsT=v_sb[:, jt, h, :],
                                rhs=at_sbs[h][:, jt, :],
                                start=(jt == 0), stop=(jt == 1),
                                tile_position=(0, 32 * a),
                                skip_group_check=True,
                            )
                    for jt in range(2):
                        for a in range(4):
                            h = 4 * ot + a
                            nc.tensor.matmul(
                                out=lden[32 * a:32 * a + D, ot, :],
                                lhsT=ones32[:],
                                rhs=at_sbs[h][:, jt, :],
                                start=(jt == 0), stop=(jt == 1),
                                tile_position=(0, 32 * a),
                                skip_group_check=True,
                            )
                    # normalize: oc = otu * (1/l), channel-major bf16
                    nc.vector.reciprocal_approx_fast(
                        out=rec_sb[:, ot, :], in_=lden[:, ot, :]
                    )
                    nc.vector.scalar_tensor_tensor(
                        out=oc_sb[:, ot, :], in0=otu[:, ot, :], scalar=OC_SCALE,
                        in1=rec_sb[:, ot, :], op0=ALU.mult, op1=ALU.mult,
                    )

                # ---- residual transpose-accumulate + Wo projection ----
                # (transposes first: they only need xu, so the PE isn't
                # stalled on the recip/mult chain producing oc)
                pt_ps = pspt.tile([128, 2, 256], F32, tag="pspt")
                for pt in range(2):
                    for ct in range(2):
                        nc.tensor.matmul(
                            out=pt_ps[:, pt, 128 * ct:128 * ct + 128].bitcast(F32R),
                            lhsT=xv[:, ct, 128 * pt:128 * pt + 128],
                            rhs=ident[:],
                            is_transpose=True, start=(ct == 0), stop=False,
                            skip_group_check=True,
                        )
                    nc.tensor.matmul(
                        out=pt_ps[:, pt, :],
                        lhsT=oc_sb[:, :, 128 * pt:128 * pt + 128],
                        rhs=wo_sb[:, :, :],
                        start=False, stop=True,
                        perf_mode=PM.DoubleRow,
                        skip_group_check=True,
                    )

                # ---- LayerNorm (free axis = channels), written
                # directly into the pixel-major out slab ----
                mv2 = p_small.tile([128, 2, 2], F32, tag="mv2")
                for pt in range(2):
                    stats = p_small.tile([128, 6], F32, tag="stats")
                    nc.vector.bn_stats(out=stats[:], in_=pt_ps[:, pt, :])
                    nc.vector.bn_aggr(out=mv2[:, pt, :], in_=stats[:])
                # rstd = 1/sqrt(var+eps) via magic-constant + two Newton steps (DVE)
                ve = p_small.tile([128, 2], F32, tag="ve")
                nc.vector.tensor_scalar(out=ve[:], in0=mv2[:, :, 1], scalar1=EPS,
                                        scalar2=None, op0=ALU.add)
                hbits = p_small.tile([128, 2], I32, tag="hbits")
                nc.vector.tensor_scalar(out=hbits[:], in0=ve[:].bitcast(I32),
                                        scalar1=1, scalar2=None, op0=ALU.arith_shift_right)
                y0 = p_small.tile([128, 2], F32, tag="y0")
                nc.vector.tensor_tensor(out=y0[:].bitcast(I32), in0=magic_sb[:],
                                        in1=hbits[:], op=ALU.subtract)
                a_t = p_small.tile([128, 2], F32, tag="a_t")
                nc.vector.tensor_tensor(out=a_t[:], in0=ve[:], in1=y0[:], op=ALU.mult)
                nc.vector.tensor_tensor(out=a_t[:], in0=a_t[:], in1=y0[:], op=ALU.mult)
                nc.vector.tensor_scalar(out=a_t[:], in0=a_t[:], scalar1=-0.5, scalar2=1.5,
                                        op0=ALU.mult, op1=ALU.add)
                rstd2 = p_small.tile([128, 2], F32, tag="rstd2")
                nc.vector.tensor_tensor(out=rstd2[:], in0=y0[:], in1=a_t[:], op=ALU.mult)
                b_t = p_small.tile([128, 2], F32, tag="b_t")
                nc.vector.tensor_tensor(out=b_t[:], in0=ve[:], in1=rstd2[:], op=ALU.mult)
                nc.vector.tensor_tensor(out=b_t[:], in0=b_t[:], in1=rstd2[:], op=ALU.mult)
                nc.vector.tensor_scalar(out=b_t[:], in0=b_t[:], scalar1=-0.5, scalar2=1.5,
                                        op0=ALU.mult, op1=ALU.add)
                nc.vector.tensor_tensor(out=rstd2[:], in0=rstd2[:], in1=b_t[:], op=ALU.mult)
                nmr2 = p_small.tile([128, 2], F32, tag="nmr2")
                nc.vector.scalar_tensor_tensor(
                    out=nmr2[:], in0=mv2[:, :, 0], scalar=-1.0, in1=rstd2[:],
                    op0=ALU.mult, op1=ALU.mult,
                )
                out_sb = slab_sbs[s]["out"]
                for pt in range(2):
                    nc.vector.tensor_scalar(
                        out=out_sb[:, j, pt, :], in0=pt_ps[:, pt, :],
                        scalar1=rstd2[:, pt:pt + 1], scalar2=nmr2[:, pt:pt + 1],
                        op0=ALU.mult, op1=ALU.add,
                    )
                    if apply_gb:
                        nc.vector.tensor_tensor(
                            out=out_sb[:, j, pt, :], in0=out_sb[:, j, pt, :],
                            in1=G128[:], op=ALU.mult
                        )
                        nc.vector.tensor_tensor(
                            out=out_sb[:, j, pt, :], in0=out_sb[:, j, pt, :],
                            in1=B128[:], op=ALU.add
                        )
                return st

            def emit_tail(st):
                pass

            total = n_slabs * n_units
            f_states = [None, None]
            for idx in range(total + 2):
                if idx < total:
                    s, j = divmod(idx, n_units)
                    if j == 0 and s == 0:
                        x8_sb = p_x8in.tile([128, 2, N_UNITS, 256], FP8, tag="x8_sb")
                        nc.sync.dma_start(
                            out=x8_sb[:],
                            in_=x8_ext[s].rearrange("(t p) u i -> p t u i", p=128),
                        )
                        x_sb = p_xin.tile([128, 2, N_UNITS, 256], F32R, tag="x_sb")
                        nc.sync.dma_start(
                            out=x_sb[:],
                            in_=x_ext[s].rearrange("(t p) u i -> p t u i", p=128),
                        )
                        slab_sbs[s] = {"x": x_sb, "x8": x8_sb}
                    if j == 0:
                        out_sb = p_xout.tile([128, N_UNITS, 2, C], F32, tag="out_sb")
                        if n_units < N_UNITS:
                            nc.vector.memset(out_sb[:], 0.0)
                        slab_sbs[s]["out"] = out_sb
                    if j == n_units // 2 and s + 1 < n_slabs:
                        x8_sb = p_x8in.tile([128, 2, N_UNITS, 256], FP8, tag="x8_sb")
                        nc.sync.dma_start(
                            out=x8_sb[:],
                            in_=x8_ext[s + 1].rearrange("(t p) u i -> p t u i", p=128),
                        )
                        x_sb = p_xin.tile([128, 2, N_UNITS, 256], F32R, tag="x_sb")
                        nc.sync.dma_start(
                            out=x_sb[:],
                            in_=x_ext[s + 1].rearrange("(t p) u i -> p t u i", p=128),
                        )
                        slab_sbs[s + 1] = {"x": x_sb, "x8": x8_sb}
                    fs = emit_frontA(s, j)
                else:
                    fs = None
                old = f_states.pop(0)
                f_states.append(fs)
                if old is not None:
                    emit_mid(old)
                    if old["j"] == n_units - 1:
                        ps_ = old["s"]
                        nc.sync.dma_start(
                            out=out_ext[ps_].rearrange("u (t p) c -> p u t c", p=128),
                            in_=slab_sbs[ps_]["out"][:],
                        )
                if fs is not None:
                    emit_frontB(fs)

    nc.compile()
    return nc


_CACHE = {}


def _get(apply_gb: bool):
    if apply_gb not in _CACHE:
        _CACHE[apply_gb] = _build(apply_gb)
    return _CACHE[apply_gb]


def _in_maps(x, Wq, Wk, Wv, Wo, gamma, beta):
    import ml_dtypes
    E4M3 = ml_dtypes.float8_e4m3fn
    x = np.ascontiguousarray(x, dtype=np.float32)
    B = x.shape[0]
    xr = x.reshape(B, C, 12, BS, 12, BS).transpose(0, 2, 1, 4, 3, 5).reshape(B * 12, C, 12, BS * BS)
    xres = np.ascontiguousarray(xr * np.float32(S_WO))
    x8 = np.ascontiguousarray((xr * np.float32(G_X)).astype(E4M3))
    wqt = np.ascontiguousarray((np.asarray(Wq, dtype=np.float32).T * np.float32(S_W)).astype(E4M3))
    wkt = np.ascontiguousarray((np.asarray(Wk, dtype=np.float32).T * np.float32(S_W)).astype(E4M3))
    wvt = np.ascontiguousarray((np.asarray(Wv, dtype=np.float32).T * np.float32(S_W)).astype(E4M3))
    wot = np.ascontiguousarray((np.asarray(Wo, dtype=np.float32).T * np.float32(S_WO)).astype(E4M3))
    g = np.ascontiguousarray(np.asarray(gamma, dtype=np.float32).reshape(1, C))
    b = np.ascontiguousarray(np.asarray(beta, dtype=np.float32).reshape(1, C))
    maps = []
    for core in range(N_CORES):
        sl = slice(core * N_SLABS, (core + 1) * N_SLABS)
        maps.append({
            "x": np.ascontiguousarray(xres[sl]),
            "x8": np.ascontiguousarray(x8[sl]),
            "wqt": wqt, "wkt": wkt, "wvt": wvt, "wot": wot,
            "gamma": g, "beta": b,
        })
    return maps


def _assemble(results, B=2):
    outs = np.stack([results[i]["out"] for i in range(N_CORES)])
    # outs: [cores, ns, 12u, 256px, C] pixel-major
    o = outs.reshape(B, 12, 12, BS, BS, C).transpose(0, 5, 1, 3, 2, 4).reshape(B, C, 12 * BS, HW_)
    return np.ascontiguousarray(o)


def run(x, Wq, Wk, Wv, Wo, gamma, beta, **spmd_kwargs):
    gamma = np.asarray(gamma, dtype=np.float32)
    beta = np.asarray(beta, dtype=np.float32)
    apply_gb = not (np.allclose(gamma, 1.0) and np.all(beta == 0.0))
    nc = _get(apply_gb)
    maps = _in_maps(x, Wq, Wk, Wv, Wo, gamma, beta)
    res = run_bass_kernel_spmd(nc, maps, core_ids=list(range(N_CORES)), **spmd_kwargs)
    return _assemble(res.results, B=np.asarray(x).shape[0]), res


def kernel(x, Wq, Wk, Wv, Wo, gamma, beta):
    out, _ = run(x, Wq, Wk, Wv, Wo, gamma, beta)
    return out


# revision 22
# speedup vs baseline: 1.2473x; 1.0184x over previous
"""AdaptiveBlockSelfAttention Trainium2 kernel (8-core SPMD, no collectives).

Problem: x[2,256,192,192]; 1x1-conv QKV projections; block-local attention
within 16x16 spatial blocks (8 heads, d=32); output projection + residual;
LayerNorm over channels.

Sharding: the 24 (batch, block-row) slabs are data-parallel -> 3 slabs/core
on 8 cores. Weights replicated. Everything else is core-local.

This core is PE-duty-cycle limited (HAM power throttle caps the tensor
engine near K=4/8 on 8-core SPMD), so the design minimizes PE busy-cycles
and keeps the in-order PE queue free of head-of-line stalls:
  - Q^T,K^T channel-major [128, 2, 256] per proj: head h = 4*ot + a sits
    at partitions [32a, 32a+32) of half ot -- a direct PSUM copy, and
    exactly the base partitions needed for 4-way row-tiled score matmuls.
  - Scores S^T (K=32) run 4 heads concurrently via tile_position=(32a, 0).
  - A^T = exp(S^T*scale) on ACT (the only ACT work besides qkt copies).
  - AV (M=32, no ones column) runs 4 heads concurrently via
    tile_position=(0, 32a); outputs for heads 4ot..4ot+3 land as the exact
    channel-major block [128ot:128ot+128] x [256 px].
  - Softmax denominators via separate col-tiled matmuls with lhsT =
    ones[128, 32]: each head's row-sum lands broadcast across its 32
    channel partitions -> one full-width reciprocal + one multiply per
    half, no per-head scalar chain, no gpsimd broadcasts.
  - Wo projection pixel-major (oc-stationary) + residual via PE
    transpose-accumulate of X into the same PSUM group (transposes first
    so the PE isn't waiting on the softmax-normalize DVE chain).
  - LayerNorm with bn_stats/bn_aggr; rstd via magic-constant Newton (DVE).
  - PE transpose back to channel-major, copy into the output slab.
  - All PSUM tiles come from one rotating 8-bank pool (tile = 1 bank).
  - Unit emission is software-pipelined: front(j+1) [QKV/scores/exp] is
    emitted before back(j) [AV/Wo/LN] so the PE queue always has
    independent work behind a stalled instruction.
"""

import numpy as np

import concourse.bacc as bacc
import concourse.tile as tile
import concourse.mybir as mybir
from concourse.bass_utils import run_bass_kernel_spmd
from concourse.masks import make_identity

F32 = mybir.dt.float32
F32R = mybir.dt.float32r
BF16 = mybir.dt.bfloat16
FP8 = mybir.dt.float8e4
I32 = mybir.dt.int32
PM = mybir.MatmulPerfMode
AF = mybir.ActivationFunctionType
ALU = mybir.AluOpType

N_CORES = 8
C = 256
HW_ = 192
BS = 16
NH = 8
D = 32
EPS = 1e-5
SCALE = float(1.0 / np.sqrt(D))
G_X = 16.0            # host scale on fp8 x
S_W = 16.0            # host scale on fp8 Wq/Wk/Wv
S_WO = 16.0           # host scale on fp8 Wo (residual x_res carries the
                      # matching 16x; LayerNorm is affine-invariant per pixel)
EXP_SCALE = float(SCALE / (G_X * S_W) ** 2)
OC_SCALE = float(1.0 / (G_X * S_W))

N_SLABS = 3          # block-rows per core
N_UNITS = 12         # 16x16 blocks per block-row


def _build(apply_gb: bool, n_slabs: int = N_SLABS, n_units: int = N_UNITS):
    nc = bacc.Bacc("TRN2", target_bir_lowering=False, debug=False)

    x_ext = nc.declare_dram_parameter("x", [n_slabs, C, N_UNITS, 256], F32R, isOutput=False)
    x8_ext = nc.declare_dram_parameter("x8", [n_slabs, C, N_UNITS, 256], FP8, isOutput=False)
    out_ext = nc.declare_dram_parameter("out", [n_slabs, N_UNITS, 256, C], F32, isOutput=True)
    wq_ext = nc.declare_dram_parameter("wqt", [C, C], FP8, isOutput=False)
    wk_ext = nc.declare_dram_parameter("wkt", [C, C], FP8, isOutput=False)
    wv_ext = nc.declare_dram_parameter("wvt", [C, C], FP8, isOutput=False)
    wo_ext = nc.declare_dram_parameter("wot", [C, C], FP8, isOutput=False)
    gamma_ext = nc.declare_dram_parameter("gamma", [1, C], F32, isOutput=False)
    beta_ext = nc.declare_dram_parameter("beta", [1, C], F32, isOutput=False)

    with tile.TileContext(nc) as tc:
        with (
            tc.tile_pool(name="consts", bufs=1) as consts,
            tc.tile_pool(name="xin", bufs=2) as p_xin,
            tc.tile_pool(name="x8in", bufs=2) as p_x8in,
            tc.tile_pool(name="xu", bufs=3) as p_xu,
            tc.tile_pool(name="xout", bufs=2) as p_xout,
            tc.tile_pool(name="qksb", bufs=3) as p_qk,
            tc.tile_pool(name="vsb", bufs=4) as p_v,
            tc.tile_pool(name="atsb", bufs=26) as p_at,
            tc.tile_pool(name="ocsb", bufs=2) as p_oc,
            tc.tile_pool(name="recsb", bufs=2) as p_rec,
            tc.tile_pool(name="usb", bufs=2) as p_u,
            tc.tile_pool(name="small", bufs=8) as p_small,
            tc.tile_pool(name="psf", bufs=4, space="PSUM") as psf,
            tc.tile_pool(name="psav", bufs=2, space="PSUM") as psav,
            tc.tile_pool(name="pspt", bufs=2, space="PSUM") as pspt,
        ):
            # ---- constants ----
            ident_f32 = consts.tile([128, 128], F32, tag="ident_f32")
            make_identity(nc, ident_f32[:])
            ident = consts.tile([128, 128], F32R)
            nc.vector.tensor_copy(out=ident[:], in_=ident_f32[:])
            ones32 = consts.tile([128, D], BF16, tag="ones32")
            nc.vector.memset(ones32[:], 1.0)
            magic_sb = consts.tile([128, 2], I32, tag="magic")
            nc.vector.memset(magic_sb[:], 0x5F3759DF)

            w_sbs = {}
            for nm, ext in (("wq", wq_ext), ("wk", wk_ext), ("wv", wv_ext), ("wo", wo_ext)):
                w_sb = consts.tile([128, 2, C], FP8, tag=nm)
                nc.sync.dma_start(out=w_sb[:], in_=ext[:].rearrange("(t p) o -> p t o", p=128))
                w_sbs[nm] = w_sb
            wq_sb, wk_sb, wv_sb, wo_sb = (w_sbs[n] for n in ("wq", "wk", "wv", "wo"))

            if apply_gb:
                g_row = consts.tile([1, C], F32, tag="g_row")
                b_row = consts.tile([1, C], F32, tag="b_row")
                nc.sync.dma_start(out=g_row[:], in_=gamma_ext[:])
                nc.sync.dma_start(out=b_row[:], in_=beta_ext[:])
                G128 = consts.tile([128, C], F32, tag="G128")
                B128 = consts.tile([128, C], F32, tag="B128")
                nc.gpsimd.partition_broadcast(out_ap=G128[:], in_ap=g_row[:])
                nc.gpsimd.partition_broadcast(out_ap=B128[:], in_ap=b_row[:])

            slab_sbs = {}

            def emit_st_round(qkt_sb, at_sbs, ot):
                st_a = []
                for _a in range(4):
                    st_t = psf.tile([128, 2, 256], F32, tag="psf")
                    st_a.append(st_t)
                for jt in range(2):
                    for a in range(4):
                        nc.tensor.matmul(
                            out=st_a[a][:, jt, :],
                            lhsT=qkt_sb[32 * a:32 * a + D, 2 + ot, 128 * jt:128 * jt + 128],
                            rhs=qkt_sb[32 * a:32 * a + D, ot, :],
                            start=True, stop=True,
                            tile_position=(32 * a, 0),
                            skip_group_check=True,
                        )
                for a in range(4):
                    h = 4 * ot + a
                    at_sb = p_at.tile([128, 2, 256], BF16, tag="at")
                    nc.scalar.activation(
                        out=at_sb[:], in_=st_a[a][:], func=AF.Exp, scale=EXP_SCALE
                    )
                    at_sbs[h] = at_sb

            def emit_frontA(s, j):
                # unit view of fp8 X: [c, kt, 256 px] contiguous
                x8v = slab_sbs[s]["x8"][:, :, j, :]

                # ---- Q^T, K^T channel-major (fp8 DoubleRow, K=256/pass);
                # head h = 4*ot + a at partitions [32a, 32a+32) of half ot ----
                qk_q = psf.tile([128, 2, 256], F32, tag="psf")
                qk_k = psf.tile([128, 2, 256], F32, tag="psf")
                for qk_ps, w_sb in ((qk_q, wq_sb), (qk_k, wk_sb)):
                    for ot in range(2):
                        nc.tensor.matmul(
                            out=qk_ps[:, ot, :],
                            lhsT=w_sb[:, :, 128 * ot:128 * ot + 128],
                            rhs=x8v[:, :, :],
                            start=True, stop=True,
                            perf_mode=PM.DoubleRow,
                        )
                qkt_sb = p_qk.tile([128, 4, 256], BF16, tag="qkt")
                nc.scalar.activation(out=qkt_sb[:, 0:2, :], in_=qk_q[:], func=AF.Copy)
                nc.scalar.activation(out=qkt_sb[:, 2:4, :], in_=qk_k[:], func=AF.Copy)

                at_sbs = [None] * NH
                emit_st_round(qkt_sb, at_sbs, 0)
                return {"s": s, "j": j, "qkt": qkt_sb, "at": at_sbs}

            def emit_frontB(st):
                s, j = st["s"], st["j"]
                emit_st_round(st["qkt"], st["at"], 1)
                # ---- V pixel-major [j, 2(jt), 8 heads, 32] (fp8 DoubleRow) ----
                x8v = slab_sbs[s]["x8"][:, :, j, :]
                v_ps = psf.tile([128, 2, 256], F32, tag="psf")
                for pt in range(2):
                    nc.tensor.matmul(
                        out=v_ps[:, pt, :],
                        lhsT=x8v[:, :, 128 * pt:128 * pt + 128],
                        rhs=wv_sb[:, :, :],
                        start=True, stop=True,
                        perf_mode=PM.DoubleRow,
                    )
                v_sb = p_v.tile([128, 2, NH, D], BF16, tag="v_sb")
                nc.vector.tensor_copy(
                    out=v_sb[:].rearrange("p t h d -> p t (h d)"), in_=v_ps[:]
                )
                st["v_sb"] = v_sb

            def emit_mid(st):
                s, j = st["s"], st["j"]
                v_sb, at_sbs = st["v_sb"], st["at"]
                xv = slab_sbs[s]["x"][:, :, j, :]

                # ---- AV (col-tiled 4-way) + denominators ----
                otu = psav.tile([128, 2, 256], F32, tag="psav")
                lden = psav.tile([128, 2, 256], F32, tag="psav")
                rec_sb = p_rec.tile([128, 2, 256], F32, tag="rec")
                oc_sb = p_oc.tile([128, 2, 256], FP8, tag="oc")
                for ot in range(2):
                    for jt in range(2):
                        for a in range(4):
                            h = 4 * ot + a
                            nc.tensor.matmul(
                                out=otu[32 * a:32 * a + D, ot, :],
                                lhsT=v_sb[:, jt, h, :],
                                rhs=at_sbs[h][:, jt, :],
                                start=(jt == 0), stop=(jt == 1),
                                tile_position=(0, 32 * a),
                                skip_group_check=True,
                            )
                    for jt in range(2):
                        for a in range(4):
                            h = 4 * ot + a
                            nc.tensor.matmul(
                                out=lden[32 * a:32 * a + D, ot, :],
                                lhsT=ones32[:],
                                rhs=at_sbs[h][:, jt, :],
                                start=(jt == 0), stop=(jt == 1),
                                tile_position=(0, 32 * a),
                                skip_group_check=True,
                            )
                    # normalize: oc = otu * (1/l), channel-major bf16
                    nc.vector.reciprocal_approx_fast(
                        out=rec_sb[:, ot, :], in_=lden[:, ot, :]
                    )
                    nc.vector.scalar_tensor_tensor(
                        out=oc_sb[:, ot, :], in0=otu[:, ot, :], scalar=OC_SCALE,
                        in1=rec_sb[:, ot, :], op0=ALU.mult, op1=ALU.mult,
                    )

                # ---- residual transpose-accumulate + Wo projection ----
                # (transposes first: they only need xu, so the PE isn't
                # stalled on the recip/mult chain producing oc)
                pt_ps = pspt.tile([128, 2, 256], F32, tag="pspt")
                for pt in range(2):
                    for ct in range(2):
                        nc.tensor.matmul(
                            out=pt_ps[:, pt, 128 * ct:128 * ct + 128].bitcast(F32R),
                            lhsT=xv[:, ct, 128 * pt:128 * pt + 128],
                            rhs=ident[:],
                            is_transpose=True, start=(ct == 0), stop=False,
                            skip_group_check=True,
                        )
                    nc.tensor.matmul(
                        out=pt_ps[:, pt, :],
                        lhsT=oc_sb[:, :, 128 * pt:128 * pt + 128],
                        rhs=wo_sb[:, :, :],
                        start=False, stop=True,
                        perf_mode=PM.DoubleRow,
                        skip_group_check=True,
                    )

                # ---- LayerNorm (free axis = channels), written
                # directly into the pixel-major out slab ----
                mv2 = p_small.tile([128, 2, 2], F32, tag="mv2")
                for pt in range(2):
                    stats = p_small.tile([128, 6], F32, tag="stats")
                    nc.vector.bn_stats(out=stats[:], in_=pt_ps[:, pt, :])
                    nc.vector.bn_aggr(out=mv2[:, pt, :], in_=stats[:])
                # rstd = 1/sqrt(var+eps) via magic-constant + two Newton steps (DVE)
                ve = p_small.tile([128, 2], F32, tag="ve")
                nc.vector.tensor_scalar(out=ve[:], in0=mv2[:, :, 1], scalar1=EPS,
                                        scalar2=None, op0=ALU.add)
                hbits = p_small.tile([128, 2], I32, tag="hbits")
                nc.vector.tensor_scalar(out=hbits[:], in0=ve[:].bitcast(I32),
                                        scalar1=1, scalar2=None, op0=ALU.arith_shift_right)
                y0 = p_small.tile([128, 2], F32, tag="y0")
                nc.vector.tensor_tensor(out=y0[:].bitcast(I32), in0=magic_sb[:],
                                        in1=hbits[:], op=ALU.subtract)
                a_t = p_small.tile([128, 2], F32, tag="a_t")
                nc.vector.tensor_tensor(out=a_t[:], in0=ve[:], in1=y0[:], op=ALU.mult)
                nc.vector.tensor_tensor(out=a_t[:], in0=a_t[:], in1=y0[:], op=ALU.mult)
                nc.vector.tensor_scalar(out=a_t[:], in0=a_t[:], scalar1=-0.5, scalar2=1.5,
                                        op0=ALU.mult, op1=ALU.add)
                rstd2 = p_small.tile([128, 2], F32, tag="rstd2")
                nc.vector.tensor_tensor(out=rstd2[:], in0=y0[:], in1=a_t[:], op=ALU.mult)
                b_t = p_small.tile([128, 2], F32, tag="b_t")
                nc.vector.tensor_tensor(out=b_t[:], in0=ve[:], in1=rstd2[:], op=ALU.mult)
                nc.vector.tensor_tensor(out=b_t[:], in0=b_t[:], in1=rstd2[:], op=ALU.mult)
                nc.vector.tensor_scalar(out=b_t[:], in0=b_t[:], scalar1=-0.5, scalar2=1.5,
                                        op0=ALU.mult, op1=ALU.add)
                nc.vector.tensor_tensor(out=rstd2[:], in0=rstd2[:], in1=b_t[:], op=ALU.mult)
                nmr2 = p_small.tile([128, 2], F32, tag="nmr2")
                nc.vector.scalar_tensor_tensor(
                    out=nmr2[:], in0=mv2[:, :, 0], scalar=-1.0, in1=rstd2[:],
                    op0=ALU.mult, op1=ALU.mult,
                )
                out_sb = slab_sbs[s]["out"]
                for pt in range(2):
                    nc.vector.tensor_scalar(
                        out=out_sb[:, j, pt, :], in0=pt_ps[:, pt, :],
                        scalar1=rstd2[:, pt:pt + 1], scalar2=nmr2[:, pt:pt + 1],
                        op0=ALU.mult, op1=ALU.add,
                    )
                    if apply_gb:
                        nc.vector.tensor_tensor(
                            out=out_sb[:, j, pt, :], in0=out_sb[:, j, pt, :],
                            in1=G128[:], op=ALU.mult
                        )
                        nc.vector.tensor_tensor(
                            out=out_sb[:, j, pt, :], in0=out_sb[:, j, pt, :],
                            in1=B128[:], op=ALU.add
                        )
                return st

            def emit_tail(st):
                pass

            total = n_slabs * n_units
            f_states = [None, None]
            for idx in range(total + 2):
                if idx < total:
                    s, j = divmod(idx, n_units)
                    if j == 0 and s == 0:
                        x8_sb = p_x8in.tile([128, 2, N_UNITS, 256], FP8, tag="x8_sb")
                        nc.sync.dma_start(
                            out=x8_sb[:],
                            in_=x8_ext[s].rearrange("(t p) u i -> p t u i", p=128),
                        )
                        x_sb = p_xin.tile([128, 2, N_UNITS, 256], F32R, tag="x_sb")
                        nc.sync.dma_start(
                            out=x_sb[:],
                            in_=x_ext[s].rearrange("(t p) u i -> p t u i", p=128),
                        )
                        slab_sbs[s] = {"x": x_sb, "x8": x8_sb}
                    if j == 0:
                        out_sb = p_xout.tile([128, N_UNITS, 2, C], F32, tag="out_sb")
                        if n_units < N_UNITS:
                            nc.vector.memset(out_sb[:], 0.0)
                        slab_sbs[s]["out"] = out_sb
                    if j == n_units // 2 and s + 1 < n_slabs:
                        x8_sb = p_x8in.tile([128, 2, N_UNITS, 256], FP8, tag="x8_sb")
                        nc.sync.dma_start(
                            out=x8_sb[:],
                            in_=x8_ext[s + 1].rearrange("(t p) u i -> p t u i", p=128),
                        )
                        x_sb = p_xin.tile([128, 2, N_UNITS, 256], F32R, tag="x_sb")
                        nc.sync.dma_start(
                            out=x_sb[:],
                            in_=x_ext[s + 1].rearrange("(t p) u i -> p t u i", p=128),
                        )
                        slab_sbs[s + 1] = {"x": x_sb, "x8": x8_sb}
                    fs = emit_frontA(s, j)
                else:
                    fs = None
                old = f_states.pop(0)
                f_states.append(fs)
                if old is not None:
                    emit_mid(old)
                    if old["j"] == n_units - 1:
                        ps_ = old["s"]
                        nc.sync.dma_start(
                            out=out_ext[ps_].rearrange("u (t p) c -> p u t c", p=128),
                            in_=slab_sbs[ps_]["out"][:],
                        )
                if fs is not None:
                    emit_frontB(fs)

    nc.compile()
    return nc


_CACHE = {}


def _get(apply_gb: bool):
    if apply_gb not in _CACHE:
        _CACHE[apply_gb] = _build(apply_gb)
    return _CACHE[apply_gb]


def _in_maps(x, Wq, Wk, Wv, Wo, gamma, beta):
    import ml_dtypes
    E4M3 = ml_dtypes.float8_e4m3fn
    x = np.ascontiguousarray(x, dtype=np.float32)
    B = x.shape[0]
    xr = x.reshape(B, C, 12, BS, 12, BS).transpose(0, 2, 1, 4, 3, 5).reshape(B * 12, C, 12, BS * BS)
    xres = np.ascontiguousarray(xr * np.float32(S_WO))
    x8 = np.ascontiguousarray((xr * np.float32(G_X)).astype(E4M3))
    wqt = np.ascontiguousarray((np.asarray(Wq, dtype=np.float32).T * np.float32(S_W)).astype(E4M3))
    wkt = np.ascontiguousarray((np.asarray(Wk, dtype=np.float32).T * np.float32(S_W)).astype(E4M3))
    wvt = np.ascontiguousarray((np.asarray(Wv, dtype=np.float32).T * np.float32(S_W)).astype(E4M3))
    wot = np.ascontiguousarray((np.asarray(Wo, dtype=np.float32).T * np.float32(S_WO)).astype(E4M3))
    g = np.ascontiguousarray(np.asarray(gamma, dtype=np.float32).reshape(1, C))
    b = np.ascontiguousarray(np.asarray(beta, dtype=np.float32).reshape(1, C))
    maps = []
    for core in range(N_CORES):
        sl = slice(core * N_SLABS, (core + 1) * N_SLABS)
        maps.append({
            "x": np.ascontiguousarray(xres[sl]),
            "x8": np.ascontiguousarray(x8[sl]),
            "wqt": wqt, "wkt": wkt, "wvt": wvt, "wot": wot,
            "gamma": g, "beta": b,
        })
    return maps


def _assemble(results, B=2):
    outs = np.stack([results[i]["out"] for i in range(N_CORES)])
    # outs: [cores, ns, 12u, 256px, C] pixel-major
    o = outs.reshape(B, 12, 12, BS, BS, C).transpose(0, 5, 1, 3, 2, 4).reshape(B, C, 12 * BS, HW_)
    return np.ascontiguousarray(o)


def run(x, Wq, Wk, Wv, Wo, gamma, beta, **spmd_kwargs):
    gamma = np.asarray(gamma, dtype=np.float32)
    beta = np.asarray(beta, dtype=np.float32)
    apply_gb = not (np.allclose(gamma, 1.0) and np.all(beta == 0.0))
    nc = _get(apply_gb)
    maps = _in_maps(x, Wq, Wk, Wv, Wo, gamma, beta)
    res = run_bass_kernel_spmd(nc, maps, core_ids=list(range(N_CORES)), **spmd_kwargs)
    return _assemble(res.results, B=np.asarray(x).shape[0]), res


def kernel(x, Wq, Wk, Wv, Wo, gamma, beta):
    out, _ = run(x, Wq, Wk, Wv, Wo, gamma, beta)
    return out
